# revision 27
# baseline (speedup 1.0000x reference)
"""ECE loss kernel for Trainium2 (Bass/Tile), data-parallel over 8 NeuronCores.

Math (per sample row of logits[N, C]):
  conf = max softmax(x) = exp(max(x)) / sum(exp(x))
  acc  = (argmax(x) == label)
  ece  = sum_b |conf_sum[b] - acc_sum[b]| / N   over 16 ceil-bins of conf

The host casts logits to bf16 (a precision choice well inside the 2e-2
tolerance: ECE here telescopes over same-sign bins, so per-sample conf
noise largely cancels) which halves HBM traffic — the hard bottleneck in
this environment (measured aggregate DMA ~165-210 GB/s deep-queue).

Device work per core (125k rows as [125 partitions x 1000 samples]):
  - DMA 22 bf16 tiles ([125, k, 100], 1.25MB) on the gpsimd/SWDGE queue
    (the only queue whose transfers rotate across all 16 SDMA engines;
    HWDGE rings are pinned to 5 engines here), 14-slot pipeline.
  - ACT: E = exp(x) in place (monotone, so row-max of E works for conf/acc)
  - PE : per-sample class sums via PSUM accumulation: 10 identity-matmuls
         per tile over 10-class slices (bf16, 1 cyc/row) -> psum[p,k,10];
         DVE finishes with a width-10 reduce. This keeps the big class-sum
         reduction off the Vector engine.
  - DVE: row-max reduce of E (the one remaining full pass), then per-chunk
         tail: sinv = recip(s), acc = (exp(g) == max E), conf = max*sinv,
         v = 2*acc + conf.
  - Binning via per-threshold accumulated sums, split across engines and
    spread between tile iterations (3 chunks: 500/250/250 samples):
      ACT: R_b = sum relu(conf - C_b), T = sum conf,
           A_b = sum sign(v - (2+C_b)), n1 = sum acc
      DVE: NN_b = sum (conf <= C_b)
  Host recovers per-bin sums:
      S_b = T - R_b - C_b*(n - NN_b)         (cum conf mass up to C_b)
      K_b = (n - A_b)/2 ; cumacc_b = K_b - n0
      conf_sum/acc_sum by first differences; ece = sum |.| / N
C_b is the exact f32 boundary: the largest f32 y with f32(15*y) <= b+1, so
binning matches the reference's ceil(conf*15) semantics.
"""

import os

import ml_dtypes
import numpy as np

import concourse.bass as bass
import concourse.mybir as mybir
import concourse.tile as tile
from concourse.bass_utils import run_bass_kernel_spmd

F32 = mybir.dt.float32
BF16 = mybir.dt.bfloat16
ALU = mybir.AluOpType
AX = mybir.AxisListType
ACTF = mybir.ActivationFunctionType

N = 1_000_000
C = 100
NCORES = 8
ROWS = N // NCORES          # 125000 rows per core
P = 125                     # SBUF partitions used
SPP = ROWS // P             # 1000 samples per partition
TILE_K = 50                 # samples per partition per tile
NBINS = 16                  # 15 real bins + always-empty tail bin
NGRP = 10                   # classes per matmul group
NCHUNK = 3                  # binning chunks (500/250/250 samples/partition)

LAST_RESULTS = None         # stashed BassKernelResults for test harness


def _bin_thresholds():
    """C_b = largest f32 y such that f32(15*y) <= b+1, for b = 0..14."""
    thr = []
    for b in range(15):
        tgt = np.float32(b + 1)

        def f(v):
            return np.float32(np.float32(15.0) * v)

        y = np.float32((b + 1) / 15.0)
        if f(y) <= tgt:
            while True:
                y2 = np.nextafter(y, np.float32(np.inf))
                if f(y2) <= tgt:
                    y = y2
                else:
                    break
        else:
            while f(y) > tgt:
                y = np.nextafter(y, np.float32(-np.inf))
        thr.append(np.float32(y))
    return thr


THR = _bin_thresholds()


def _build():
    nc = bass.Bass(trn_type="TRN2")
    x = nc.dram_tensor("x", [P, SPP * C], BF16, kind="ExternalInput")
    g = nc.dram_tensor("g", [P, SPP], BF16, kind="ExternalInput")
    eye = nc.dram_tensor("eye", [P, P], F32, kind="ExternalInput")
    thr = nc.dram_tensor("thr", [P, 32], F32, kind="ExternalInput")
    r_out = nc.dram_tensor("r", [P, NCHUNK * NBINS], F32, kind="ExternalOutput")
    nn_out = nc.dram_tensor("nn", [P, NCHUNK * NBINS], F32, kind="ExternalOutput")
    a_out = nc.dram_tensor("a", [P, NCHUNK * NBINS], F32, kind="ExternalOutput")

    X = x[:, :].rearrange("p (k c) -> p k c", c=C)  # [125, 1000, 100]

    sizes = [12, 13, 25] + [TILE_K] * 19
    assert sum(sizes) == SPP
    CHUNK_LO = [0, 500, 800]
    CHUNK_HI = [500, 800, 1000]
    BUFS = 14

    with tile.TileContext(nc) as tc:
        with (
            tc.tile_pool(name="xin", bufs=BUFS) as xin,
            tc.tile_pool(name="persist", bufs=1) as persist,
            tc.psum_pool(name="ps", bufs=4) as ps,
        ):
            em_bf = persist.tile([P, SPP], BF16)
            em_all = persist.tile([P, SPP], F32)
            s_all = persist.tile([P, SPP], F32)
            g_sb = persist.tile([P, SPP], BF16)
            eg_bf = persist.tile([P, SPP], BF16)
            acc_sb = persist.tile([P, SPP], F32)
            eye_sb = persist.tile([P, P], F32)
            eye_bf = persist.tile([P, P], BF16)
            r_sb = persist.tile([P, NCHUNK * NBINS], F32)
            nn_sb = persist.tile([P, NCHUNK * NBINS], F32)
            a_sb = persist.tile([P, NCHUNK * NBINS], F32)
            act_scr = persist.tile([P, 500], F32)
            dve_scr = persist.tile([P, 500], F32)
            thr_sb = persist.tile([P, 32], F32)
            pacer = persist.tile([P, 6900], mybir.dt.bfloat16)
            # pairwise-max tree scratch (bf16, zero-padded tails: E > 0 so
            # zero pads never win a max)
            m1 = persist.tile([P, TILE_K, 56], BF16)
            m2 = persist.tile([P, TILE_K, 32], BF16)
            m3 = persist.tile([P, TILE_K, 16], BF16)
            m4 = persist.tile([P, TILE_K, 8], BF16)
            m5 = persist.tile([P, TILE_K, 4], BF16)
            m6 = persist.tile([P, TILE_K, 2], BF16)
            nc.vector.memset(m1[:, :, :], 0.0)
            nc.vector.memset(m2[:, :, :], 0.0)
            nc.sync.dma_start(out=eye_sb[:, :], in_=eye[:, :])
            nc.sync.dma_start(out=thr_sb[:, :], in_=thr[:, :])
            nc.sync.dma_start(out=g_sb[:, :], in_=g[:, :])
            nc.scalar.activation(eye_bf[:, :], eye_sb[:, :], ACTF.Copy)

            pending = []  # deferred binning closures, drained between tiles

            def tail_prep(c):
                cs = slice(CHUNK_LO[c], CHUNK_HI[c])
                n_c = CHUNK_HI[c] - CHUNK_LO[c]
                nc.vector.reciprocal(s_all[:, cs], s_all[:, cs])
                # exp(g) through the same bf16 spline/rounding as the tile
                # exps, so the accuracy equality-compare is bit-exact when
                # the label hits the argmax
                nc.scalar.activation(eg_bf[:, cs], g_sb[:, cs], ACTF.Exp)
                # em to f32 for the conf arithmetic (exact widening)
                nc.scalar.activation(em_all[:, cs], em_bf[:, cs], ACTF.Copy)
                nc.vector.tensor_tensor(
                    acc_sb[:, cs], eg_bf[:, cs], em_bf[:, cs], op=ALU.is_equal
                )
                nc.vector.tensor_tensor(
                    em_all[:, cs], em_all[:, cs], s_all[:, cs], op=ALU.mult
                )
                conf = em_all[:, cs]
                for b in range(15):
                    cb = float(THR[b])

                    def op_r(b=b, c=c, conf=conf, n_c=n_c):
                        nc.scalar.activation(
                            act_scr[:, :n_c], conf, ACTF.Relu,
                            bias=thr_sb[:, b : b + 1],
                            accum_out=r_sb[:, c * NBINS + b : c * NBINS + b + 1],
                        )

                    def op_nn(b=b, c=c, cb=cb, conf=conf, n_c=n_c):
                        nc.vector.tensor_scalar(
                            dve_scr[:, :n_c], conf, cb, None,
                            op0=ALU.is_le, op1=ALU.add,
                            accum_out=nn_sb[:, c * NBINS + b : c * NBINS + b + 1],
                        )

                    def op_a(b=b, c=c, cb=cb, cs=cs, conf=conf, n_c=n_c):
                        nc.vector.scalar_tensor_tensor(
                            dve_scr[:, :n_c], conf, cb, acc_sb[:, cs],
                            op0=ALU.is_le, op1=ALU.mult,
                            accum_out=a_sb[:, c * NBINS + b : c * NBINS + b + 1],
                        )

                    pending.append(op_r)
                    pending.append(op_nn)
                    pending.append(op_a)

                def op_t(c=c, conf=conf, n_c=n_c):
                    nc.scalar.activation(
                        act_scr[:, :n_c], conf, ACTF.Identity, bias=0.0,
                        accum_out=r_sb[:, c * NBINS + 15 : c * NBINS + 16],
                    )

                def op_n1(c=c, cs=cs, n_c=n_c):
                    nc.scalar.activation(
                        act_scr[:, :n_c], acc_sb[:, cs], ACTF.Identity, bias=0.0,
                        accum_out=a_sb[:, c * NBINS + 15 : c * NBINS + 16],
                    )

                pending.append(op_t)
                pending.append(op_n1)

            def drain_pending(nops):
                for _ in range(nops):
                    if not pending:
                        return
                    pending.pop(0)()

            off = 0
            for t, k in enumerate(sizes):
                sl = slice(off, off + k)
                off += k
                xt = xin.tile([P, TILE_K, C], BF16, tag="xt")
                if t >= 3:
                    # open-loop pacing: ~5us of dummy Pool work between
                    # dispatches keeps a few transfers in flight -- deep
                    # queues put the SDMA engines in a slow regime and
                    # starve completion-semaphore delivery (measured)
                    nc.gpsimd.memset(pacer[:, :], 0.0)
                nc.gpsimd.dma_start(out=xt[:, :k, :], in_=X[:, sl, :])
                # E = exp(x) in place (bf16): row-max is over E (monotone)
                # and the accuracy compare uses the same spline output
                nc.scalar.activation(xt[:, :k, :], xt[:, :k, :], ACTF.Exp)
                # per-sample class sums: 10-class slices accumulated in PSUM
                pt = ps.tile([P, TILE_K, NGRP], F32, tag="ps")
                for gi in range(C // NGRP):
                    nc.tensor.matmul(
                        pt[:, :k, :],
                        eye_bf[:, :],
                        xt[:, :k, gi * NGRP : (gi + 1) * NGRP],
                        start=(gi == 0),
                        stop=(gi == C // NGRP - 1),
                    )
                # rsum first: it frees the PSUM bank quickly; rmax is long
                nc.vector.reduce_sum(out=s_all[:, sl], in_=pt[:, :k, :], axis=AX.X)
                # row-max of E via a bf16 pairwise-max tree: tensor_tensor
                # max runs at 2 elem/cycle on bf16 (vs 1x for tensor_reduce)
                TT = nc.vector.tensor_tensor
                TT(m1[:, :k, 0:50], xt[:, :k, 0:50], xt[:, :k, 50:100], op=ALU.max)
                TT(m2[:, :k, 0:28], m1[:, :k, 0:28], m1[:, :k, 28:56], op=ALU.max)
                TT(m3[:, :k, 0:16], m2[:, :k, 0:16], m2[:, :k, 16:32], op=ALU.max)
                TT(m4[:, :k, 0:8], m3[:, :k, 0:8], m3[:, :k, 8:16], op=ALU.max)
                TT(m5[:, :k, 0:4], m4[:, :k, 0:4], m4[:, :k, 4:8], op=ALU.max)
                TT(m6[:, :k, 0:2], m5[:, :k, 0:2], m5[:, :k, 2:4], op=ALU.max)
                em3 = em_bf[:, sl].rearrange("p (k o) -> p k o", o=1)
                TT(em3, m6[:, :k, 0:1], m6[:, :k, 1:2], op=ALU.max)
                if off in (500, 800):
                    tail_prep((500, 800).index(off))
                drain_pending(8)
            tail_prep(2)
            drain_pending(len(pending))

            nc.sync.dma_start(out=r_out[:, :], in_=r_sb[:, :])
            nc.sync.dma_start(out=nn_out[:, :], in_=nn_sb[:, :])
            nc.sync.dma_start(out=a_out[:, :], in_=a_sb[:, :])

    import bass_rust as _br

    # Instructions carry at most 2 sync commands (waits + completion update),
    # so any instruction the Tile scheduler gave >1 wait has its extra waits
    # peeled onto same-engine drains inserted just before it.
    for bb in nc.m.functions[0].blocks:
        while True:
            insns = list(bb.instructions)
            target = None
            for idx, ins in enumerate(insns):
                si = ins.sync_info
                if si is None:
                    continue
                if len(si.on_wait) > 1:
                    target = (idx, ins)
                    break
            if target is None:
                break
            idx, ins = target
            si = ins.sync_info
            waits = list(si.on_wait)
            if type(ins).__name__ == "InstDrain":
                room = max(0, 1 - len(si.on_update))
            else:
                room = 1
            keep, extra = waits[len(waits) - room :], waits[: len(waits) - room]
            pos = idx
            for i, w in enumerate(extra):
                nd = mybir.InstDrain(
                    name=f"{ins.name}-presync{i}", ins=[], outs=[],
                    bass_is_fusable=False,
                )
                nd.engine = ins.engine
                nd.sync_info = _br.SyncInfo(on_wait=[w], on_update=[])
                nc.register_instruction(nd, overwrite=True)
                bb.instructions.insert(pos, nd)
                pos += 1
            si.on_wait = keep
            ins.sync_info = si
    return nc


_NC_CACHE = {}


def _get_nc():
    if "nc" not in _NC_CACHE:
        _NC_CACHE["nc"] = _build()
    return _NC_CACHE["nc"]


def kernel(logits, labels):
    global LAST_RESULTS
    logits = np.asarray(logits)
    labels_i = np.asarray(labels).astype(np.int64)
    assert logits.shape == (N, C), logits.shape

    # bf16 precision choice: halves HBM traffic (the bottleneck); the label
    # logit is gathered AFTER the cast so its bits match x exactly
    x_bf = logits.astype(ml_dtypes.bfloat16)
    g_bf = x_bf[np.arange(N), labels_i]

    eye = np.eye(P, dtype=np.float32)
    thr_cols = np.zeros(32, dtype=np.float32)
    for b in range(15):
        thr_cols[b] = -THR[b]
        thr_cols[15 + b] = -np.float32(np.float64(2.0) + np.float64(THR[b]))
    thr_arr = np.broadcast_to(thr_cols, (P, 32)).copy()

    in_maps = []
    for c in range(NCORES):
        sl = slice(c * ROWS, (c + 1) * ROWS)
        in_maps.append(
            {
                "x": x_bf[sl].reshape(P, SPP * C),
                "g": g_bf[sl].reshape(P, SPP),
                "eye": eye,
                "thr": thr_arr,
            }
        )

    trace = bool(int(os.environ.get("ECE_TRACE", "0")))
    res = run_bass_kernel_spmd(
        _get_nc(), in_maps, core_ids=list(range(NCORES)), trace=trace
    )
    LAST_RESULTS = res

    r = np.zeros(NCHUNK * NBINS, np.float64)
    nn_ = np.zeros(NCHUNK * NBINS, np.float64)
    a = np.zeros(NCHUNK * NBINS, np.float64)
    for out in res.results:
        r += out["r"].astype(np.float64).sum(axis=0)
        nn_ += out["nn"].astype(np.float64).sum(axis=0)
        a += out["a"].astype(np.float64).sum(axis=0)
    r = r.reshape(NCHUNK, NBINS).sum(axis=0)
    nn_ = nn_.reshape(NCHUNK, NBINS).sum(axis=0)
    a = a.reshape(NCHUNK, NBINS).sum(axis=0)

    thr64 = np.array([np.float64(t) for t in THR])
    T = r[15]
    n1 = a[15]
    S = T - r[:15] - thr64 * (N - nn_[:15])
    cumconf = np.concatenate([S, [T]])
    conf_sum = np.diff(cumconf, prepend=0.0)
    cumacc = np.concatenate([a[:15], [n1]])
    acc_sum = np.diff(cumacc, prepend=0.0)
    ece = np.abs(conf_sum - acc_sum).sum() / N
    return np.array([ece], dtype=np.float32)


# revision 28
# speedup vs baseline: 1.1366x; 1.1366x over previous
"""ECE loss kernel for Trainium2 (Bass/Tile), data-parallel over 8 NeuronCores.

Math (per sample row of logits[N, C]):
  conf = max softmax(x) = exp(max(x)) / sum(exp(x))
  acc  = (argmax(x) == label)
  ece  = sum_b |conf_sum[b] - acc_sum[b]| / N   over 16 ceil-bins of conf

The host casts logits to bf16 (a precision choice well inside the 2e-2
tolerance: ECE here telescopes over same-sign bins, so per-sample conf
noise largely cancels) which halves HBM traffic — the hard bottleneck in
this environment (measured aggregate DMA ~165-210 GB/s deep-queue).

Device work per core (125k rows as [125 partitions x 1000 samples]):
  - DMA 22 bf16 tiles ([125, k, 100], 1.25MB) on the gpsimd/SWDGE queue
    (the only queue whose transfers rotate across all 16 SDMA engines;
    HWDGE rings are pinned to 5 engines here), 14-slot pipeline.
  - ACT: E = exp(x) in place (monotone, so row-max of E works for conf/acc)
  - PE : per-sample class sums via PSUM accumulation: 10 identity-matmuls
         per tile over 10-class slices (bf16, 1 cyc/row) -> psum[p,k,10];
         DVE finishes with a width-10 reduce. This keeps the big class-sum
         reduction off the Vector engine.
  - DVE: row-max reduce of E (the one remaining full pass), then per-chunk
         tail: sinv = recip(s), acc = (exp(g) == max E), conf = max*sinv,
         v = 2*acc + conf.
  - Binning via per-threshold accumulated sums, split across engines and
    spread between tile iterations (3 chunks: 500/250/250 samples):
      ACT: R_b = sum relu(conf - C_b), T = sum conf,
           A_b = sum sign(v - (2+C_b)), n1 = sum acc
      DVE: NN_b = sum (conf <= C_b)
  Host recovers per-bin sums:
      S_b = T - R_b - C_b*(n - NN_b)         (cum conf mass up to C_b)
      K_b = (n - A_b)/2 ; cumacc_b = K_b - n0
      conf_sum/acc_sum by first differences; ece = sum |.| / N
C_b is the exact f32 boundary: the largest f32 y with f32(15*y) <= b+1, so
binning matches the reference's ceil(conf*15) semantics.
"""

import os

import ml_dtypes
import numpy as np

import concourse.bass as bass
import concourse.mybir as mybir
import concourse.tile as tile
from concourse.bass_utils import run_bass_kernel_spmd

F32 = mybir.dt.float32
BF16 = mybir.dt.bfloat16
ALU = mybir.AluOpType
AX = mybir.AxisListType
ACTF = mybir.ActivationFunctionType

N = 1_000_000
C = 100
NCORES = 8
ROWS = N // NCORES          # 125000 rows per core
P = 125                     # SBUF partitions used
SPP = ROWS // P             # 1000 samples per partition
TILE_K = 50                 # samples per partition per tile
NBINS = 16                  # 15 real bins + always-empty tail bin
NGRP = 10                   # classes per matmul group
NCHUNK = 3                  # binning chunks (500/250/250 samples/partition)

LAST_RESULTS = None         # stashed BassKernelResults for test harness


def _bin_thresholds():
    """C_b = largest f32 y such that f32(15*y) <= b+1, for b = 0..14."""
    thr = []
    for b in range(15):
        tgt = np.float32(b + 1)

        def f(v):
            return np.float32(np.float32(15.0) * v)

        y = np.float32((b + 1) / 15.0)
        if f(y) <= tgt:
            while True:
                y2 = np.nextafter(y, np.float32(np.inf))
                if f(y2) <= tgt:
                    y = y2
                else:
                    break
        else:
            while f(y) > tgt:
                y = np.nextafter(y, np.float32(-np.inf))
        thr.append(np.float32(y))
    return thr


THR = _bin_thresholds()


def _build():
    nc = bass.Bass(trn_type="TRN2")
    x = nc.dram_tensor("x", [P, SPP * C], BF16, kind="ExternalInput")
    g = nc.dram_tensor("g", [P, SPP], BF16, kind="ExternalInput")
    eye = nc.dram_tensor("eye", [P, P], F32, kind="ExternalInput")
    thr = nc.dram_tensor("thr", [P, 32], F32, kind="ExternalInput")
    r_out = nc.dram_tensor("r", [P, NCHUNK * NBINS], F32, kind="ExternalOutput")
    nn_out = nc.dram_tensor("nn", [P, NCHUNK * NBINS], F32, kind="ExternalOutput")
    a_out = nc.dram_tensor("a", [P, NCHUNK * NBINS], F32, kind="ExternalOutput")

    X = x[:, :].rearrange("p (k c) -> p k c", c=C)  # [125, 1000, 100]

    sizes = [12, 13, 25] + [TILE_K] * 19
    assert sum(sizes) == SPP
    CHUNK_LO = [0, 500, 800]
    CHUNK_HI = [500, 800, 1000]
    BUFS = 14

    with tile.TileContext(nc) as tc:
        with (
            tc.tile_pool(name="xin", bufs=BUFS) as xin,
            tc.tile_pool(name="persist", bufs=1) as persist,
            tc.psum_pool(name="ps", bufs=4) as ps,
        ):
            em_bf = persist.tile([P, SPP], BF16)
            em_all = persist.tile([P, SPP], F32)
            s_all = persist.tile([P, SPP], F32)
            g_sb = persist.tile([P, SPP], BF16)
            eg_bf = persist.tile([P, SPP], BF16)
            acc_sb = persist.tile([P, SPP], F32)
            eye_sb = persist.tile([P, P], F32)
            eye_bf = persist.tile([P, P], BF16)
            r_sb = persist.tile([P, NCHUNK * NBINS], F32)
            nn_sb = persist.tile([P, NCHUNK * NBINS], F32)
            a_sb = persist.tile([P, NCHUNK * NBINS], F32)
            act_scr = persist.tile([P, 500], F32)
            dve_scr = persist.tile([P, 500], F32)
            thr_sb = persist.tile([P, 32], F32)
            pacer = persist.tile([P, 5760], mybir.dt.bfloat16)
            # pairwise-max tree scratch (bf16, zero-padded tails: E > 0 so
            # zero pads never win a max)
            m1 = persist.tile([P, TILE_K, 56], BF16)
            m2 = persist.tile([P, TILE_K, 32], BF16)
            m3 = persist.tile([P, TILE_K, 16], BF16)
            m4 = persist.tile([P, TILE_K, 8], BF16)
            m5 = persist.tile([P, TILE_K, 4], BF16)
            m6 = persist.tile([P, TILE_K, 2], BF16)
            nc.vector.memset(m1[:, :, :], 0.0)
            nc.vector.memset(m2[:, :, :], 0.0)
            nc.sync.dma_start(out=eye_sb[:, :], in_=eye[:, :])
            nc.sync.dma_start(out=thr_sb[:, :], in_=thr[:, :])
            nc.sync.dma_start(out=g_sb[:, :], in_=g[:, :])
            nc.scalar.activation(eye_bf[:, :], eye_sb[:, :], ACTF.Copy)

            pending = []  # deferred binning closures, drained between tiles

            def tail_prep(c):
                cs = slice(CHUNK_LO[c], CHUNK_HI[c])
                n_c = CHUNK_HI[c] - CHUNK_LO[c]
                nc.vector.reciprocal(s_all[:, cs], s_all[:, cs])
                # exp(g) through the same bf16 spline/rounding as the tile
                # exps, so the accuracy equality-compare is bit-exact when
                # the label hits the argmax
                nc.scalar.activation(eg_bf[:, cs], g_sb[:, cs], ACTF.Exp)
                # em to f32 for the conf arithmetic (exact widening)
                nc.scalar.activation(em_all[:, cs], em_bf[:, cs], ACTF.Copy)
                nc.vector.tensor_tensor(
                    acc_sb[:, cs], eg_bf[:, cs], em_bf[:, cs], op=ALU.is_equal
                )
                nc.vector.tensor_tensor(
                    em_all[:, cs], em_all[:, cs], s_all[:, cs], op=ALU.mult
                )
                conf = em_all[:, cs]
                for b in range(15):
                    cb = float(THR[b])

                    def op_r(b=b, c=c, conf=conf, n_c=n_c):
                        nc.scalar.activation(
                            act_scr[:, :n_c], conf, ACTF.Relu,
                            bias=thr_sb[:, b : b + 1],
                            accum_out=r_sb[:, c * NBINS + b : c * NBINS + b + 1],
                        )

                    def op_nn(b=b, c=c, cb=cb, conf=conf, n_c=n_c):
                        nc.vector.tensor_scalar(
                            dve_scr[:, :n_c], conf, cb, None,
                            op0=ALU.is_le, op1=ALU.add,
                            accum_out=nn_sb[:, c * NBINS + b : c * NBINS + b + 1],
                        )

                    def op_a(b=b, c=c, cb=cb, cs=cs, conf=conf, n_c=n_c):
                        nc.vector.scalar_tensor_tensor(
                            dve_scr[:, :n_c], conf, cb, acc_sb[:, cs],
                            op0=ALU.is_le, op1=ALU.mult,
                            accum_out=a_sb[:, c * NBINS + b : c * NBINS + b + 1],
                        )

                    pending.append(op_r)
                    pending.append(op_nn)
                    pending.append(op_a)

                def op_t(c=c, conf=conf, n_c=n_c):
                    nc.scalar.activation(
                        act_scr[:, :n_c], conf, ACTF.Identity, bias=0.0,
                        accum_out=r_sb[:, c * NBINS + 15 : c * NBINS + 16],
                    )

                def op_n1(c=c, cs=cs, n_c=n_c):
                    nc.scalar.activation(
                        act_scr[:, :n_c], acc_sb[:, cs], ACTF.Identity, bias=0.0,
                        accum_out=a_sb[:, c * NBINS + 15 : c * NBINS + 16],
                    )

                pending.append(op_t)
                pending.append(op_n1)

            def drain_pending(nops):
                for _ in range(nops):
                    if not pending:
                        return
                    pending.pop(0)()

            off = 0
            for t, k in enumerate(sizes):
                sl = slice(off, off + k)
                off += k
                xt = xin.tile([P, TILE_K, C], BF16, tag="xt")
                if t >= 3:
                    # open-loop pacing: ~5us of dummy Pool work between
                    # dispatches keeps a few transfers in flight -- deep
                    # queues put the SDMA engines in a slow regime and
                    # starve completion-semaphore delivery (measured)
                    nc.gpsimd.memset(pacer[:, :], 0.0)
                nc.gpsimd.dma_start(out=xt[:, :k, :], in_=X[:, sl, :])
                # E = exp(x) in place (bf16): row-max is over E (monotone)
                # and the accuracy compare uses the same spline output
                nc.scalar.activation(xt[:, :k, :], xt[:, :k, :], ACTF.Exp)
                # per-sample class sums: 10-class slices accumulated in PSUM
                pt = ps.tile([P, TILE_K, NGRP], F32, tag="ps")
                for gi in range(C // NGRP):
                    nc.tensor.matmul(
                        pt[:, :k, :],
                        eye_bf[:, :],
                        xt[:, :k, gi * NGRP : (gi + 1) * NGRP],
                        start=(gi == 0),
                        stop=(gi == C // NGRP - 1),
                    )
                # rsum first: it frees the PSUM bank quickly; rmax is long
                nc.vector.reduce_sum(out=s_all[:, sl], in_=pt[:, :k, :], axis=AX.X)
                # row-max of E via a bf16 pairwise-max tree: tensor_tensor
                # max runs at 2 elem/cycle on bf16 (vs 1x for tensor_reduce)
                TT = nc.vector.tensor_tensor
                TT(m1[:, :k, 0:50], xt[:, :k, 0:50], xt[:, :k, 50:100], op=ALU.max)
                TT(m2[:, :k, 0:28], m1[:, :k, 0:28], m1[:, :k, 28:56], op=ALU.max)
                TT(m3[:, :k, 0:16], m2[:, :k, 0:16], m2[:, :k, 16:32], op=ALU.max)
                TT(m4[:, :k, 0:8], m3[:, :k, 0:8], m3[:, :k, 8:16], op=ALU.max)
                TT(m5[:, :k, 0:4], m4[:, :k, 0:4], m4[:, :k, 4:8], op=ALU.max)
                TT(m6[:, :k, 0:2], m5[:, :k, 0:2], m5[:, :k, 2:4], op=ALU.max)
                em3 = em_bf[:, sl].rearrange("p (k o) -> p k o", o=1)
                TT(em3, m6[:, :k, 0:1], m6[:, :k, 1:2], op=ALU.max)
                if off in (500, 800):
                    tail_prep((500, 800).index(off))
                drain_pending(8)
            tail_prep(2)
            drain_pending(len(pending))

            nc.sync.dma_start(out=r_out[:, :], in_=r_sb[:, :])
            nc.sync.dma_start(out=nn_out[:, :], in_=nn_sb[:, :])
            nc.sync.dma_start(out=a_out[:, :], in_=a_sb[:, :])

    import bass_rust as _br

    # Instructions carry at most 2 sync commands (waits + completion update),
    # so any instruction the Tile scheduler gave >1 wait has its extra waits
    # peeled onto same-engine drains inserted just before it.
    for bb in nc.m.functions[0].blocks:
        while True:
            insns = list(bb.instructions)
            target = None
            for idx, ins in enumerate(insns):
                si = ins.sync_info
                if si is None:
                    continue
                if len(si.on_wait) > 1:
                    target = (idx, ins)
                    break
            if target is None:
                break
            idx, ins = target
            si = ins.sync_info
            waits = list(si.on_wait)
            if type(ins).__name__ == "InstDrain":
                room = max(0, 1 - len(si.on_update))
            else:
                room = 1
            keep, extra = waits[len(waits) - room :], waits[: len(waits) - room]
            pos = idx
            for i, w in enumerate(extra):
                nd = mybir.InstDrain(
                    name=f"{ins.name}-presync{i}", ins=[], outs=[],
                    bass_is_fusable=False,
                )
                nd.engine = ins.engine
                nd.sync_info = _br.SyncInfo(on_wait=[w], on_update=[])
                nc.register_instruction(nd, overwrite=True)
                bb.instructions.insert(pos, nd)
                pos += 1
            si.on_wait = keep
            ins.sync_info = si
    return nc


_NC_CACHE = {}


def _get_nc():
    if "nc" not in _NC_CACHE:
        _NC_CACHE["nc"] = _build()
    return _NC_CACHE["nc"]


def kernel(logits, labels):
    global LAST_RESULTS
    logits = np.asarray(logits)
    labels_i = np.asarray(labels).astype(np.int64)
    assert logits.shape == (N, C), logits.shape

    # bf16 precision choice: halves HBM traffic (the bottleneck); the label
    # logit is gathered AFTER the cast so its bits match x exactly
    x_bf = logits.astype(ml_dtypes.bfloat16)
    g_bf = x_bf[np.arange(N), labels_i]

    eye = np.eye(P, dtype=np.float32)
    thr_cols = np.zeros(32, dtype=np.float32)
    for b in range(15):
        thr_cols[b] = -THR[b]
        thr_cols[15 + b] = -np.float32(np.float64(2.0) + np.float64(THR[b]))
    thr_arr = np.broadcast_to(thr_cols, (P, 32)).copy()

    in_maps = []
    for c in range(NCORES):
        sl = slice(c * ROWS, (c + 1) * ROWS)
        in_maps.append(
            {
                "x": x_bf[sl].reshape(P, SPP * C),
                "g": g_bf[sl].reshape(P, SPP),
                "eye": eye,
                "thr": thr_arr,
            }
        )

    trace = bool(int(os.environ.get("ECE_TRACE", "0")))
    res = run_bass_kernel_spmd(
        _get_nc(), in_maps, core_ids=list(range(NCORES)), trace=trace
    )
    LAST_RESULTS = res

    r = np.zeros(NCHUNK * NBINS, np.float64)
    nn_ = np.zeros(NCHUNK * NBINS, np.float64)
    a = np.zeros(NCHUNK * NBINS, np.float64)
    for out in res.results:
        r += out["r"].astype(np.float64).sum(axis=0)
        nn_ += out["nn"].astype(np.float64).sum(axis=0)
        a += out["a"].astype(np.float64).sum(axis=0)
    r = r.reshape(NCHUNK, NBINS).sum(axis=0)
    nn_ = nn_.reshape(NCHUNK, NBINS).sum(axis=0)
    a = a.reshape(NCHUNK, NBINS).sum(axis=0)

    thr64 = np.array([np.float64(t) for t in THR])
    T = r[15]
    n1 = a[15]
    S = T - r[:15] - thr64 * (N - nn_[:15])
    cumconf = np.concatenate([S, [T]])
    conf_sum = np.diff(cumconf, prepend=0.0)
    cumacc = np.concatenate([a[:15], [n1]])
    acc_sum = np.diff(cumacc, prepend=0.0)
    ece = np.abs(conf_sum - acc_sum).sum() / N
    return np.array([ece], dtype=np.float32)
